# revision 62
# baseline (speedup 1.0000x reference)
"""DocRE model kernel for 8 Trainium2 NeuronCores.

Data-parallel over the pair grid: core = b*4 + ib owns document b and
i-rows [8*ib, 8*ib+8) of the 32x32 entity-pair grid (256 pairs/core).
All weights are replicated; W_ext (49152x768, repacked partition-major
on the host) is streamed from HBM in 3MB chunks through fp16 matmuls
against group-bilinear tiles built on-chip.  The hs/ts factors are
round-tripped through DRAM so per-group partition-replicated layouts
(hsdup / tsd) can be produced by plain DMAs instead of PE broadcasts.
"""

import numpy as np

import concourse.bacc as bacc
import concourse.bass as bass
import concourse.tile as tile
from concourse import mybir
from concourse.bass_utils import run_bass_kernel_spmd
from concourse.masks import make_identity

F32 = mybir.dt.float32
F16 = mybir.dt.float16

B, L, H = 2, 1024, 768
E, M = 32, 4
EMB, BLK, NL = 768, 64, 97
G = EMB // BLK  # 12
LN_EPS = 1e-12

N_CORES = 8
IB = E // (N_CORES // B)     # 8 i-rows per core
NPAIR = IB * E               # 256 pairs per core
PT = NPAIR // 128            # 2 pair-tiles
KT = EMB * BLK // 128        # 384 k-tiles
WCH = 8                      # k-tiles per W_ext DMA chunk (1.5 MB each)
BKT = 4                      # k-tiles per DVE bilinear batch
CT = EMB // 128              # 6 feature chunks
KC = H // 128                # 6 contraction chunks of H
LC = L // 128                # 8 chunks of L
NENT = IB + E + 1            # 41 cols: [my 8 entities | all 32 | cls]
NE2 = NENT + 1


def _build_module():
    nc = bacc.Bacc("TRN2", target_bir_lowering=False, debug=False)

    seq_d = nc.dram_tensor("seq", [L, H], F16, kind="ExternalInput")
    S_d = nc.dram_tensor("S", [L, NENT], F16, kind="ExternalInput")
    Wh_d = nc.dram_tensor("Wh", [3 * H, EMB], F16, kind="ExternalInput")
    Wt_d = nc.dram_tensor("Wt", [3 * H, EMB], F16, kind="ExternalInput")
    bh_d = nc.dram_tensor("bh", [128, CT], F32, kind="ExternalInput")
    bt_d = nc.dram_tensor("bt", [128, CT], F32, kind="ExternalInput")
    # W_ext repacked host-side to partition-major [p, kt, o] so each chunk
    # DMA is one fully-contiguous line per partition.
    Wx_d = nc.dram_tensor("Wx", [128, KT * EMB], F16, kind="ExternalInput")
    E8_d = nc.dram_tensor("E8", [16, 128], F16, kind="ExternalInput")
    bx_d = nc.dram_tensor("bx", [128, EMB], F32, kind="ExternalInput")
    lng_d = nc.dram_tensor("lng", [128, EMB], F32, kind="ExternalInput")
    lnb_d = nc.dram_tensor("lnb", [128, EMB], F32, kind="ExternalInput")
    Wc_d = nc.dram_tensor("Wc", [EMB, NL], F16, kind="ExternalInput")
    out_d = nc.dram_tensor("out", [NPAIR, NL], F32, kind="ExternalOutput")

    with tile.TileContext(nc) as tc:
        with (
            tc.tile_pool(name="persist", bufs=1) as persist,
            tc.tile_pool(name="seqp", bufs=1) as seqp,
            tc.tile_pool(name="whp", bufs=2) as whp,
            tc.tile_pool(name="wxp", bufs=6) as wxp,
            tc.tile_pool(name="blp", bufs=4) as blp,
            tc.tile_pool(name="hsdupp", bufs=2) as hsdupp,
            tc.tile_pool(name="tsdp", bufs=2) as tsdp,
            tc.tile_pool(name="tsup", bufs=2) as tsup,
            tc.tile_pool(name="hstp", bufs=3) as hstp,
            tc.tile_pool(name="tmpp", bufs=2) as tmpp,
            tc.tile_pool(name="cnp", bufs=1) as cnp,
            tc.tile_pool(name="dramp", bufs=1, space="DRAM") as dramp,
            tc.tile_pool(name="psf", bufs=1, space="PSUM") as psf,
            tc.tile_pool(name="psg", bufs=3, space="PSUM") as psg,
        ):
            ident = persist.tile([128, 128], F32, name="ident")
            make_identity(nc, ident[:])

            # ---- per-column constants broadcast to all partitions ----
            bx_b = persist.tile([128, EMB], F32, name="bx_b")
            lng_b = persist.tile([128, EMB], F32, name="lng_b")
            lnb_b = persist.tile([128, EMB], F32, name="lnb_b")
            for tile_, src in ((bx_b, bx_d), (lng_b, lng_d), (lnb_b, lnb_d)):
                nc.sync.dma_start(tile_[:], src.ap())

            eps_t = persist.tile([128, 1], F32, name="eps")
            nc.vector.memset(eps_t[:], LN_EPS)

            # per-partition bias chunks bh/bt: [128, CT]
            bh_t = persist.tile([128, CT], F32, name="bh_t")
            bt_t = persist.tile([128, CT], F32, name="bt_t")
            for tile_, src in ((bh_t, bh_d), (bt_t, bt_d)):
                nc.sync.dma_start(tile_[:], src.ap())

            # ---- phase E: entity pooling  ent = S^T @ seq ----
            seq_t = seqp.tile([128, LC, H], F16, name="seq_t")
            S_t = seqp.tile([128, LC, NENT], F16, name="S_t")
            seq_re = seq_d.ap().rearrange("(c p) h -> p c h", p=128)
            S_re = S_d.ap().rearrange("(c p) n -> p c n", p=128)
            nc.scalar.dma_start(S_t[:], S_re)
            nc.sync.dma_start(seq_t[:, 0:LC // 2, :], seq_re[:, 0:LC // 2, :])
            nc.sync.dma_start(seq_t[:, LC // 2:LC, :], seq_re[:, LC // 2:LC, :])

            ps_e0 = psg.tile([NENT, 512], F32, name="gen")
            ps_e1 = psg.tile([NENT, 256], F32, name="gen")
            for kc in range(LC):
                nc.tensor.matmul(ps_e0[:], S_t[:, kc, :], seq_t[:, kc, 0:512],
                                 start=(kc == 0), stop=(kc == LC - 1))
                nc.tensor.matmul(ps_e1[:], S_t[:, kc, :], seq_t[:, kc, 512:768],
                                 start=(kc == 0), stop=(kc == LC - 1))
            ent_nat = persist.tile([NENT, H], F32, name="ent_nat")
            nc.vector.tensor_scalar_mul(ent_nat[:, 0:512], ps_e0[:], 1.0)
            nc.vector.tensor_scalar_mul(ent_nat[:, 512:768], ps_e1[:], 1.0)

            # transpose ent -> entT [h, NENT]  (feeds the projection matmuls)
            entT = persist.tile([128, KC, NENT], F16, name="entT")
            for kc in range(KC):
                ps_tr = psg.tile([128, NENT], F32, name="gen")
                nc.tensor.transpose(ps_tr[:], ent_nat[:, kc * 128:(kc + 1) * 128],
                                    ident[:NENT, :NENT])
                nc.vector.tensor_scalar_mul(entT[:, kc, :], ps_tr[:], 1.0)

            # ---- phase A: A/B/C projections ----
            # ABCD[ct][:, m, :] ([c, 41], m: Ah,Bh,At,Bt) built from per-ct
            # mini-chains so group 0 of phase M can start after ct=0 only.
            ABCD = []
            for ct in range(CT):
                ABCD.append(persist.tile([128, 4, NE2], F32, name=f"abcd{ct}"))

            ps_feat = [[psf.tile([128, 512], F32, name=f"pf{pt}a"),
                        psf.tile([128, 256], F32, name=f"pf{pt}b")]
                       for pt in range(PT)]

            # the six projection weight blocks, resident through the head,
            # loaded in first-use order (ts side before hs side).
            W4 = [None] * 4
            WC = {}

            def _load_wblock(w_d, blk, name):
                w4 = persist.tile([128, KC, EMB], F16, name=name)
                nc.sync.dma_start(
                    w4[:], w_d.ap()[blk * H:(blk + 1) * H, :].rearrange(
                        "(c p) h -> p c h", p=128))
                return w4

            WC[1] = _load_wblock(Wt_d, 2, "wc_ts")
            W4[2] = _load_wblock(Wt_d, 0, "w4_2")
            W4[3] = _load_wblock(Wt_d, 1, "w4_3")
            WC[0] = _load_wblock(Wh_d, 2, "wc_hs")
            W4[0] = _load_wblock(Wh_d, 0, "w4_0")
            W4[1] = _load_wblock(Wh_d, 1, "w4_1")

            def emit_ab_mini(m, ct):
                # ABCD[ct][:, m, :NENT] = (ent @ W4[m][:, :, ct*128:...]).T
                ps_n = psg.tile([NENT, 128], F32, name="gen")
                for kc in range(KC):
                    nc.tensor.matmul(ps_n[:], entT[:, kc, :],
                                     W4[m][:, kc, ct * 128:(ct + 1) * 128],
                                     start=(kc == 0), stop=(kc == KC - 1))
                x_n = tmpp.tile([NENT, 128], F32, name="x_n")
                nc.vector.tensor_scalar_mul(x_n[:], ps_n[:], 1.0)
                ps_tr = psg.tile([128, NENT], F32, name="gen")
                nc.tensor.transpose(ps_tr[:], x_n[:], ident[:NENT, :NENT])
                nc.vector.tensor_scalar_mul(ABCD[ct][:, m, 0:NENT], ps_tr[:], 1.0)

            c_dram = dramp.tile([2, EMB], F32, name="c_dram")
            CB = {}

            def emit_c_chain(side, bias_t):
                # C = cls @ W[2H:3H] + b, spread to [128, CT] via DRAM
                ps_c0 = psg.tile([NENT, 512], F32, name="gen")
                ps_c1 = psg.tile([NENT, 256], F32, name="gen")
                w_t = WC[side]
                for kc in range(KC):
                    nc.tensor.matmul(ps_c0[:1, :], entT[:, kc, IB + E:IB + E + 1],
                                     w_t[:, kc, 0:512],
                                     start=(kc == 0), stop=(kc == KC - 1))
                    nc.tensor.matmul(ps_c1[:1, :], entT[:, kc, IB + E:IB + E + 1],
                                     w_t[:, kc, 512:768],
                                     start=(kc == 0), stop=(kc == KC - 1))
                c_nat = cnp.tile([1, EMB], F32, name="c_nat")
                nc.vector.tensor_scalar_mul(c_nat[:, 0:512], ps_c0[:1, :], 1.0)
                nc.vector.tensor_scalar_mul(c_nat[:, 512:768], ps_c1[:1, :], 1.0)
                nc.scalar.dma_start(c_dram[side:side + 1, :], c_nat[:])
                cbr = tmpp.tile([128, CT], F32, name="cbr")
                c_rd = bass.AP(tensor=c_dram.tensor,
                               offset=c_dram.offset + side * EMB,
                               ap=[[1, 128], [128, CT]])
                nc.scalar.dma_start(cbr[:], c_rd)
                cb = persist.tile([128, CT], F32, name=f"cb{side}")
                nc.vector.tensor_tensor(cb[:], cbr[:], bias_t[:],
                                        op=mybir.AluOpType.add)
                CB[side] = cb

            def colview(tile_, m, col0, ap_pat):
                return bass.AP(tensor=tile_.tensor,
                               offset=tile_.offset + m * NE2 + col0,
                               ap=[tile_.ap[0]] + ap_pat)

            # DRAM round-trip buffers for the pair factors.  k-tiles tile a
            # group's 64x64 (i,j) grid as [8 di x 16 pj] so hs replicates
            # 16x and ts 8x (24x total vs 66x for [2 x 64]).
            # ts_dram: natural feature order [768 rows, 256].
            # hs_dram: digit-swapped order: row (g*64 + di*8 + ib2) holds
            #   hs feature (g*64 + ib2*8 + di) -> the 8 rows a (g, di)
            #   read needs are one contiguous 4KB block.
            ts_dram = dramp.tile([EMB, 256], F16, name="ts_dram")
            hs_dram = dramp.tile([EMB, 256], F16, name="hs_dram")

            def emit_tanh(ct, ma, mb, cola, colb, side, dst_dram, dup_order):
                # x[p, il, j] = A[p, cola+?] + B[p, ?]  (see colview patterns)
                tmp = tmpp.tile([128, 8, 32], F32, name="tmp")
                nc.vector.tensor_tensor(
                    tmp[:], colview(ABCD[ct], ma, cola[0], cola[1]),
                    colview(ABCD[ct], mb, colb[0], colb[1]),
                    op=mybir.AluOpType.add)
                xt = hstp.tile([128, 256], F16, name="xt")
                nc.scalar.activation(
                    xt[:].rearrange("p (a b) -> p a b", a=8),
                    tmp[:], mybir.ActivationFunctionType.Tanh,
                    bias=CB[side][:, ct:ct + 1], scale=1.0)
                if dup_order:
                    # partition p = ph*64 + ib2*8 + di (feature ct*128+p)
                    # -> row g*64 + di*8 + ib2 with g = 2ct + ph; one DMA
                    # per ph half keeps the dst AP at 3 dims.
                    for ph in range(2):
                        dst = bass.AP(
                            tensor=dst_dram.tensor,
                            offset=dst_dram.offset + (ct * 128 + ph * 64) * 256,
                            ap=[[256, 8], [8 * 256, 8], [1, 256]])
                        nc.scalar.dma_start(dst, xt[ph * 64:(ph + 1) * 64, :])
                else:
                    nc.scalar.dma_start(dst_dram[ct * 128:(ct + 1) * 128, :], xt[:])

            # ---- projections (six chains, PE-dense) ----

            # classifier weights + fp16 identity staged early so phase L
            # has no DMA dependency at the tail.
            wc_t = persist.tile([128, CT, NL], F16, name="wc_t")
            nc.sync.dma_start(wc_t[:], Wc_d.ap().rearrange("(c p) n -> p c n", p=128))
            ident16 = persist.tile([128, 128], F16, name="ident16")
            nc.scalar.copy(ident16[:], ident[:])

            def emit_tanh_ts(ct):
                # ts[pair=(il,j)] = tanh(At[j] + Bt[il] + Ct + bt)
                emit_tanh(ct, 2, 3, (IB, [[0, 8], [1, 32]]), (0, [[1, 8], [0, 32]]),
                          1, ts_dram, dup_order=False)

            def emit_tanh_hs(ct):
                # hs[pair=(il,j)] = tanh(Ah[il] + Bh[j] + Ch + bh)
                emit_tanh(ct, 0, 1, (0, [[1, 8], [0, 32]]), (IB, [[0, 8], [1, 32]]),
                          0, hs_dram, dup_order=True)

            # per-ct chain piece schedule: slot s of an even group emits one
            # piece for ct_next so PE/ACT/DVE work stays spread out.
            def emit_ct_piece(ct, s):
                if s == 0:
                    emit_ab_mini(2, ct)
                elif s == 1:
                    emit_ab_mini(3, ct)
                elif s == 2:
                    emit_tanh_ts(ct)
                elif s == 3:
                    emit_ab_mini(0, ct)
                elif s == 4:
                    emit_ab_mini(1, ct)
                elif s == 5:
                    emit_tanh_hs(ct)

            # head: ts-side C chain + ts pieces first, hs side after
            emit_c_chain(1, bt_t)
            for s in range(3):
                emit_ct_piece(0, s)
            emit_c_chain(0, bh_t)
            for s in range(3, 6):
                emit_ct_piece(0, s)

            # ---- phase M: main contraction over W_ext ----
            # k-tile (g, ib2, jb): partition p = pj*8 + di covers k-row
            # g*4096 + (ib2*8+di)*64 + jb*16 + pj.  Factor tiles per group:
            # hsdup[p, ib2, pair] = hs[g*64 + ib2*8 + p%8, pair]
            #   (8-partition base loaded once, then log-doubled in SBUF)
            # tsdup[p, jb, pair]  = ts[g*64 + jb*16 + p//8, pair]
            #   (16 unique rows -> 128 partitions via one tiny PE matmul)
            E8_t = persist.tile([16, 128], F16, name="E8_t")
            nc.sync.dma_start(E8_t[:], E8_d.ap())

            def emit_hs_base(hsdup, g):
                # hsdup[di, ib2, :] = hs feature g*64 + ib2*8 + di
                # (hs_dram row g*64 + di*8 + ib2)
                src = bass.AP(
                    tensor=hs_dram.tensor,
                    offset=hs_dram.offset + g * 64 * 256,
                    ap=[[8 * 256, 8], [256, 8], [1, 256]])
                nc.scalar.dma_start(hsdup[0:8, :, :], src)

            def emit_hs_double(hsdup, step):
                n = 8 << step
                nc.scalar.dma_start(hsdup[n:2 * n, :, :], hsdup[0:n, :, :])

            def emit_tsu_load(tsu, g):
                src = bass.AP(
                    tensor=ts_dram.tensor,
                    offset=ts_dram.offset + g * 64 * 256,
                    ap=[[256, 16], [16 * 256, 4], [1, 256]])
                nc.sync.dma_start(tsu[:], src)

            def emit_ts_bc(tsdup, tsu, half):
                ps_bc = psg.tile([128, 512], F32, name="gen")
                nc.tensor.matmul(ps_bc[:], E8_t[:],
                                 tsu[:, 2 * half:2 * half + 2, :].rearrange(
                                     "r j c -> r (j c)"),
                                 start=True, stop=True)
                nc.vector.tensor_scalar_mul(
                    tsdup[:, 2 * half:2 * half + 2, :].rearrange(
                        "p j c -> p (j c)"), ps_bc[:], 1.0)

            def alloc_group():
                return (hsdupp.tile([128, 8, 256], F16, name="hsdup"),
                        tsdp.tile([128, 4, 256], F16, name="tsdup"),
                        tsup.tile([16, 4, 256], F16, name="tsu"))

            cur = alloc_group()
            emit_hs_base(cur[0], 0)
            for st in range(4):
                emit_hs_double(cur[0], st)
            emit_tsu_load(cur[2], 0)
            for half in range(2):
                emit_ts_bc(cur[1], cur[2], half)

            wx_ch = None
            for g in range(G):
                nxt = alloc_group() if g + 1 < G else None
                ct_next = g // 2 + 1
                for ib2 in range(8):
                    # software-pipelined staging for group g+1
                    if nxt is not None:
                        if ib2 == 0:
                            emit_hs_base(nxt[0], g + 1)
                            emit_tsu_load(nxt[2], g + 1)
                        elif ib2 < 5:
                            emit_hs_double(nxt[0], ib2 - 1)
                        elif ib2 < 7:
                            emit_ts_bc(nxt[1], nxt[2], ib2 - 5)
                    # chain pieces for the next ct (even groups only)
                    if g % 2 == 0 and ct_next < CT and ib2 < 6:
                        emit_ct_piece(ct_next, ib2)

                    kt0 = g * 32 + ib2 * 4
                    bl4 = blp.tile([128, 4, 256], F16, name="bl4")
                    hs_b = bass.AP(tensor=cur[0].tensor,
                                   offset=cur[0].offset + ib2 * 256,
                                   ap=[cur[0].ap[0], [0, 4], [1, 256]])
                    nc.vector.tensor_tensor(bl4[:], hs_b, cur[1][:],
                                            op=mybir.AluOpType.mult)
                    for jb in range(4):
                        kt = kt0 + jb
                        if kt % WCH == 0:
                            wx_ch = wxp.tile([128, WCH * EMB], F16, name="wx_ch")
                            nc.sync.dma_start(
                                wx_ch[:], Wx_d.ap()[:, kt * EMB:(kt + WCH) * EMB])
                        kl = kt % WCH
                        for pt in range(PT):
                            lhsT = bl4[:, jb, pt * 128:(pt + 1) * 128]
                            nc.tensor.matmul(
                                ps_feat[pt][0][:], lhsT,
                                wx_ch[:, kl * EMB:kl * EMB + 512],
                                start=(kt == 0), stop=(kt == KT - 1))
                            nc.tensor.matmul(
                                ps_feat[pt][1][:], lhsT,
                                wx_ch[:, kl * EMB + 512:(kl + 1) * EMB],
                                start=(kt == 0), stop=(kt == KT - 1))
                cur = nxt

            # ---- phase L: bias, relu, layernorm, classifier ----
            for pt in range(PT):
                feat = persist.tile([128, EMB], F32, name=f"feat{pt}")
                nc.vector.tensor_tensor(feat[:, 0:512], ps_feat[pt][0][:],
                                        bx_b[:, 0:512], op=mybir.AluOpType.add)
                nc.vector.tensor_tensor(feat[:, 512:768], ps_feat[pt][1][:],
                                        bx_b[:, 512:768], op=mybir.AluOpType.add)
                nc.scalar.activation(feat[:], feat[:],
                                     mybir.ActivationFunctionType.Relu,
                                     bias=0.0, scale=1.0)

                stats = tmpp.tile([128, 3, 6], F32, name="stats")
                f_re = feat.rearrange("p (c f) -> p c f", c=3)
                for c in range(3):
                    nc.vector.bn_stats(stats[:, c, :], f_re[:, c, :])
                mv = tmpp.tile([128, 2], F32, name="mv")
                nc.vector.bn_aggr(mv[:], stats[:])
                sd = tmpp.tile([128, 1], F32, name="sd")
                nc.scalar.activation(sd[:], mv[:, 1:2],
                                     mybir.ActivationFunctionType.Sqrt,
                                     bias=eps_t[:], scale=1.0)
                rstd = tmpp.tile([128, 1], F32, name="rstd")
                nc.vector.reciprocal(rstd[:], sd[:])

                nc.vector.tensor_scalar(feat[:], feat[:], mv[:, 0:1], rstd[:],
                                        op0=mybir.AluOpType.subtract,
                                        op1=mybir.AluOpType.mult)
                nc.vector.tensor_tensor(feat[:], feat[:], lng_b[:],
                                        op=mybir.AluOpType.mult)
                ln = persist.tile([128, EMB], F16, name=f"ln{pt}")
                nc.vector.tensor_tensor(ln[:], feat[:], lnb_b[:],
                                        op=mybir.AluOpType.add)

                lnT = persist.tile([128, CT, 128], F16, name=f"lnT{pt}")
                for ct in range(CT):
                    ps_tr2 = psg.tile([128, 128], F16, name="gen")
                    nc.tensor.transpose(ps_tr2[:], ln[:, ct * 128:(ct + 1) * 128],
                                        ident16[:])
                    nc.scalar.copy(lnT[:, ct, :], ps_tr2[:])

                ps_lg = psg.tile([128, NL], F32, name="gen")
                for ct in range(CT):
                    nc.tensor.matmul(ps_lg[:], lnT[:, ct, :], wc_t[:, ct, :],
                                     start=(ct == 0), stop=(ct == CT - 1))
                out_sb = tmpp.tile([128, NL], F32, name="out_sb")
                nc.scalar.copy(out_sb[:], ps_lg[:])
                nc.scalar.dma_start(out_d.ap()[pt * 128:(pt + 1) * 128, :], out_sb[:])

    nc.compile()
    return nc


_NC_CACHE = []

# E8[r, p] = 1 iff p//8 == r: broadcasts 16 ts rows to 128 partitions
_E8 = np.zeros((16, 128), np.float16)
for _r in range(16):
    _E8[_r, _r * 8:(_r + 1) * 8] = 1.0


def _get_module():
    if not _NC_CACHE:
        _NC_CACHE.append(_build_module())
    return _NC_CACHE[0]


def _build_inputs(seq, starts, ends, mention_mask, W_head, b_head, W_tail, b_tail,
                  W_ext, b_ext, ln_g, ln_b, W_cls):
    seq = np.asarray(seq, np.float32)
    starts = np.asarray(starts, np.int64)
    ends = np.asarray(ends, np.int64)
    mask = np.asarray(mention_mask, np.float32)

    # per-document entity selection matrix: ent = Sb^T @ seq[b]
    S_b = np.zeros((B, L, E), np.float32)
    denom = np.maximum(mask.sum(axis=2), 1.0)          # [B, E]
    w = mask * 0.5 / denom[:, :, None]                 # [B, E, M]
    for b in range(B):
        for e in range(E):
            np.add.at(S_b[b, :, e], starts[b, e] + 1, w[b, e])
            np.add.at(S_b[b, :, e], ends[b, e], w[b, e])

    cls_col = np.zeros((L, 1), np.float32)
    cls_col[0, 0] = 1.0

    shared = {
        "Wh": np.ascontiguousarray(np.asarray(W_head, np.float32).astype(np.float16)),
        "Wt": np.ascontiguousarray(np.asarray(W_tail, np.float32).astype(np.float16)),
        "bh": np.ascontiguousarray(np.asarray(b_head, np.float32).reshape(CT, 128).T),
        "bt": np.ascontiguousarray(np.asarray(b_tail, np.float32).reshape(CT, 128).T),
        # partition p = pj*8+di, kt = (g, ib2, jb); row k = g*4096 +
        # (ib2*8+di)*64 + jb*16 + pj
        "Wx": np.ascontiguousarray(
            np.asarray(W_ext).astype(np.float16)
            .reshape(G, 8, 8, 4, 16, EMB).transpose(4, 2, 0, 1, 3, 5)
            .reshape(128, KT * EMB)),
        "E8": _E8,
        "bx": np.ascontiguousarray(np.broadcast_to(np.asarray(b_ext, np.float32), (128, EMB))),
        "lng": np.ascontiguousarray(np.broadcast_to(np.asarray(ln_g, np.float32), (128, EMB))),
        "lnb": np.ascontiguousarray(np.broadcast_to(np.asarray(ln_b, np.float32), (128, EMB))),
        "Wc": np.ascontiguousarray(np.asarray(W_cls, np.float32).astype(np.float16)),
    }
    in_maps = []
    for core in range(N_CORES):
        b, ib = core // 4, core % 4
        S_core = np.concatenate(
            [S_b[b][:, ib * IB:(ib + 1) * IB], S_b[b], cls_col], axis=1)
        in_maps.append({
            "seq": np.ascontiguousarray(seq[b].astype(np.float16)),
            "S": np.ascontiguousarray(S_core.astype(np.float16)),
            **shared,
        })
    return in_maps


def kernel(**inputs) -> np.ndarray:
    nc = _get_module()
    in_maps = _build_inputs(**inputs)
    res = run_bass_kernel_spmd(nc, in_maps, core_ids=list(range(N_CORES)))
    outs = np.stack([res.results[c]["out"] for c in range(N_CORES)])  # [8,256,97]
    return outs.reshape(B, 4, IB, E, NL).reshape(B, E, E, NL)


# revision 67
# speedup vs baseline: 1.1301x; 1.1301x over previous
"""DocRE model kernel for 8 Trainium2 NeuronCores.

Data-parallel over the pair grid: core = b*4 + ib owns document b and
i-rows [8*ib, 8*ib+8) of the 32x32 entity-pair grid (256 pairs/core).
All weights are replicated; W_ext (49152x768, repacked partition-major
on the host) is streamed from HBM in 3MB chunks through fp16 matmuls
against group-bilinear tiles built on-chip.  The hs/ts factors are
round-tripped through DRAM so per-group partition-replicated layouts
(hsdup / tsd) can be produced by plain DMAs instead of PE broadcasts.
"""

import numpy as np

import concourse.bacc as bacc
import concourse.bass as bass
import concourse.tile as tile
from concourse import mybir
from concourse.bass_utils import run_bass_kernel_spmd
from concourse.masks import make_identity

F32 = mybir.dt.float32
F16 = mybir.dt.float16

B, L, H = 2, 1024, 768
E, M = 32, 4
EMB, BLK, NL = 768, 64, 97
G = EMB // BLK  # 12
LN_EPS = 1e-12

N_CORES = 8
IB = E // (N_CORES // B)     # 8 i-rows per core
NPAIR = IB * E               # 256 pairs per core
PT = NPAIR // 128            # 2 pair-tiles
KT = EMB * BLK // 128        # 384 k-tiles
WCH = 8                      # k-tiles per W_ext DMA chunk (1.5 MB each)
BKT = 4                      # k-tiles per DVE bilinear batch
CT = EMB // 128              # 6 feature chunks
KC = H // 128                # 6 contraction chunks of H
LC = L // 128                # 8 chunks of L
NENT = IB + E + 1            # 41 cols: [my 8 entities | all 32 | cls]
NE2 = NENT + 1


def _build_module():
    nc = bacc.Bacc("TRN2", target_bir_lowering=False, debug=False)

    seq_d = nc.dram_tensor("seq", [L, H], F16, kind="ExternalInput")
    S_d = nc.dram_tensor("S", [L, NENT], F16, kind="ExternalInput")
    Wh_d = nc.dram_tensor("Wh", [3 * H, EMB], F16, kind="ExternalInput")
    Wt_d = nc.dram_tensor("Wt", [3 * H, EMB], F16, kind="ExternalInput")
    bh_d = nc.dram_tensor("bh", [128, CT], F32, kind="ExternalInput")
    bt_d = nc.dram_tensor("bt", [128, CT], F32, kind="ExternalInput")
    # W_ext repacked host-side to partition-major [p, kt, o] so each chunk
    # DMA is one fully-contiguous line per partition.
    Wx_d = nc.dram_tensor("Wx", [128, KT * EMB], F16, kind="ExternalInput")
    bx_d = nc.dram_tensor("bx", [128, EMB], F32, kind="ExternalInput")
    lng_d = nc.dram_tensor("lng", [128, EMB], F32, kind="ExternalInput")
    lnb_d = nc.dram_tensor("lnb", [128, EMB], F32, kind="ExternalInput")
    Wc_d = nc.dram_tensor("Wc", [EMB, NL], F16, kind="ExternalInput")
    out_d = nc.dram_tensor("out", [NPAIR, NL], F32, kind="ExternalOutput")

    with tile.TileContext(nc) as tc:
        with (
            tc.tile_pool(name="persist", bufs=1) as persist,
            tc.tile_pool(name="seqp", bufs=1) as seqp,
            tc.tile_pool(name="whp", bufs=2) as whp,
            tc.tile_pool(name="wxp", bufs=6) as wxp,
            tc.tile_pool(name="blp", bufs=4) as blp,
            tc.tile_pool(name="hsdupp", bufs=2) as hsdupp,
            tc.tile_pool(name="tsdp", bufs=2) as tsdp,
            tc.tile_pool(name="hstp", bufs=3) as hstp,
            tc.tile_pool(name="tmpp", bufs=2) as tmpp,
            tc.tile_pool(name="cnp", bufs=1) as cnp,
            tc.tile_pool(name="dramp", bufs=1, space="DRAM") as dramp,
            tc.tile_pool(name="psf", bufs=1, space="PSUM") as psf,
            tc.tile_pool(name="psg", bufs=3, space="PSUM") as psg,
        ):
            ident = persist.tile([128, 128], F32, name="ident")
            make_identity(nc, ident[:])

            # ---- per-column constants broadcast to all partitions ----
            bx_b = persist.tile([128, EMB], F32, name="bx_b")
            lng_b = persist.tile([128, EMB], F32, name="lng_b")
            lnb_b = persist.tile([128, EMB], F32, name="lnb_b")
            for tile_, src in ((bx_b, bx_d), (lng_b, lng_d), (lnb_b, lnb_d)):
                nc.sync.dma_start(tile_[:], src.ap())

            eps_t = persist.tile([128, 1], F32, name="eps")
            nc.vector.memset(eps_t[:], LN_EPS)

            # per-partition bias chunks bh/bt: [128, CT]
            bh_t = persist.tile([128, CT], F32, name="bh_t")
            bt_t = persist.tile([128, CT], F32, name="bt_t")
            for tile_, src in ((bh_t, bh_d), (bt_t, bt_d)):
                nc.sync.dma_start(tile_[:], src.ap())

            # ---- phase E: entity pooling  ent = S^T @ seq ----
            seq_t = seqp.tile([128, LC, H], F16, name="seq_t")
            S_t = seqp.tile([128, LC, NENT], F16, name="S_t")
            seq_re = seq_d.ap().rearrange("(c p) h -> p c h", p=128)
            S_re = S_d.ap().rearrange("(c p) n -> p c n", p=128)
            nc.scalar.dma_start(S_t[:], S_re)
            nc.sync.dma_start(seq_t[:, 0:LC // 2, :], seq_re[:, 0:LC // 2, :])
            nc.sync.dma_start(seq_t[:, LC // 2:LC, :], seq_re[:, LC // 2:LC, :])

            ps_e0 = psg.tile([NENT, 512], F32, name="gen")
            ps_e1 = psg.tile([NENT, 256], F32, name="gen")
            for kc in range(LC):
                nc.tensor.matmul(ps_e0[:], S_t[:, kc, :], seq_t[:, kc, 0:512],
                                 start=(kc == 0), stop=(kc == LC - 1))
                nc.tensor.matmul(ps_e1[:], S_t[:, kc, :], seq_t[:, kc, 512:768],
                                 start=(kc == 0), stop=(kc == LC - 1))
            ent_nat = persist.tile([NENT, H], F32, name="ent_nat")
            nc.vector.tensor_scalar_mul(ent_nat[:, 0:512], ps_e0[:], 1.0)
            nc.vector.tensor_scalar_mul(ent_nat[:, 512:768], ps_e1[:], 1.0)

            # transpose ent -> entT [h, NENT]  (feeds the projection matmuls)
            entT = persist.tile([128, KC, NENT], F16, name="entT")
            for kc in range(KC):
                ps_tr = psg.tile([128, NENT], F32, name="gen")
                nc.tensor.transpose(ps_tr[:], ent_nat[:, kc * 128:(kc + 1) * 128],
                                    ident[:NENT, :NENT])
                nc.vector.tensor_scalar_mul(entT[:, kc, :], ps_tr[:], 1.0)

            # ---- phase A: A/B/C projections ----
            # ABCD[ct][:, m, :] ([c, 41], m: Ah,Bh,At,Bt) built from per-ct
            # mini-chains so group 0 of phase M can start after ct=0 only.
            ABCD = []
            for ct in range(CT):
                ABCD.append(persist.tile([128, 4, NE2], F32, name=f"abcd{ct}"))

            ps_feat = [[psf.tile([128, 512], F32, name=f"pf{pt}a"),
                        psf.tile([128, 256], F32, name=f"pf{pt}b")]
                       for pt in range(PT)]

            # the six projection weight blocks, resident through the head,
            # loaded in first-use order (ts side before hs side).
            W4 = [None] * 4
            WC = {}

            def _load_wblock(w_d, blk, name):
                w4 = persist.tile([128, KC, EMB], F16, name=name)
                nc.sync.dma_start(
                    w4[:], w_d.ap()[blk * H:(blk + 1) * H, :].rearrange(
                        "(c p) h -> p c h", p=128))
                return w4

            WC[1] = _load_wblock(Wt_d, 2, "wc_ts")
            W4[2] = _load_wblock(Wt_d, 0, "w4_2")
            W4[3] = _load_wblock(Wt_d, 1, "w4_3")
            WC[0] = _load_wblock(Wh_d, 2, "wc_hs")
            W4[0] = _load_wblock(Wh_d, 0, "w4_0")
            W4[1] = _load_wblock(Wh_d, 1, "w4_1")

            def emit_ab_mini(m, ct):
                # ABCD[ct][:, m, :NENT] = (ent @ W4[m][:, :, ct*128:...]).T
                ps_n = psg.tile([NENT, 128], F32, name="gen")
                for kc in range(KC):
                    nc.tensor.matmul(ps_n[:], entT[:, kc, :],
                                     W4[m][:, kc, ct * 128:(ct + 1) * 128],
                                     start=(kc == 0), stop=(kc == KC - 1))
                x_n = tmpp.tile([NENT, 128], F32, name="x_n")
                nc.vector.tensor_scalar_mul(x_n[:], ps_n[:], 1.0)
                ps_tr = psg.tile([128, NENT], F32, name="gen")
                nc.tensor.transpose(ps_tr[:], x_n[:], ident[:NENT, :NENT])
                nc.vector.tensor_scalar_mul(ABCD[ct][:, m, 0:NENT], ps_tr[:], 1.0)

            c_dram = dramp.tile([2, EMB], F32, name="c_dram")
            CB = {}

            def emit_c_chain(side, bias_t):
                # C = cls @ W[2H:3H] + b, spread to [128, CT] via DRAM
                ps_c0 = psg.tile([NENT, 512], F32, name="gen")
                ps_c1 = psg.tile([NENT, 256], F32, name="gen")
                w_t = WC[side]
                for kc in range(KC):
                    nc.tensor.matmul(ps_c0[:1, :], entT[:, kc, IB + E:IB + E + 1],
                                     w_t[:, kc, 0:512],
                                     start=(kc == 0), stop=(kc == KC - 1))
                    nc.tensor.matmul(ps_c1[:1, :], entT[:, kc, IB + E:IB + E + 1],
                                     w_t[:, kc, 512:768],
                                     start=(kc == 0), stop=(kc == KC - 1))
                c_nat = cnp.tile([1, EMB], F32, name="c_nat")
                nc.vector.tensor_scalar_mul(c_nat[:, 0:512], ps_c0[:1, :], 1.0)
                nc.vector.tensor_scalar_mul(c_nat[:, 512:768], ps_c1[:1, :], 1.0)
                nc.scalar.dma_start(c_dram[side:side + 1, :], c_nat[:])
                cbr = tmpp.tile([128, CT], F32, name="cbr")
                c_rd = bass.AP(tensor=c_dram.tensor,
                               offset=c_dram.offset + side * EMB,
                               ap=[[1, 128], [128, CT]])
                nc.scalar.dma_start(cbr[:], c_rd)
                cb = persist.tile([128, CT], F32, name=f"cb{side}")
                nc.vector.tensor_tensor(cb[:], cbr[:], bias_t[:],
                                        op=mybir.AluOpType.add)
                CB[side] = cb

            def colview(tile_, m, col0, ap_pat):
                return bass.AP(tensor=tile_.tensor,
                               offset=tile_.offset + m * NE2 + col0,
                               ap=[tile_.ap[0]] + ap_pat)

            # DRAM round-trip buffers for the pair factors.  k-tiles tile a
            # group's 64x64 (i,j) grid as [8 di x 16 pj] so hs replicates
            # 16x and ts 8x (24x total vs 66x for [2 x 64]).
            # ts_dram: natural feature order [768 rows, 256].
            # hs_dram: digit-swapped order: row (g*64 + di*8 + ib2) holds
            #   hs feature (g*64 + ib2*8 + di) -> the 8 rows a (g, di)
            #   read needs are one contiguous 4KB block.
            ts_dram = dramp.tile([EMB, 256], F16, name="ts_dram")
            hs_dram = dramp.tile([EMB, 256], F16, name="hs_dram")

            def emit_tanh(ct, ma, mb, cola, colb, side, dst_dram, dup_order):
                # x[p, il, j] = A[p, cola+?] + B[p, ?]  (see colview patterns)
                tmp = tmpp.tile([128, 8, 32], F32, name="tmp")
                nc.vector.tensor_tensor(
                    tmp[:], colview(ABCD[ct], ma, cola[0], cola[1]),
                    colview(ABCD[ct], mb, colb[0], colb[1]),
                    op=mybir.AluOpType.add)
                xt = hstp.tile([128, 256], F16, name="xt")
                nc.scalar.activation(
                    xt[:].rearrange("p (a b) -> p a b", a=8),
                    tmp[:], mybir.ActivationFunctionType.Tanh,
                    bias=CB[side][:, ct:ct + 1], scale=1.0)
                if dup_order:
                    # partition p = ph*64 + ib2*8 + di (feature ct*128+p)
                    # -> row g*64 + di*8 + ib2 with g = 2ct + ph; one DMA
                    # per ph half keeps the dst AP at 3 dims.
                    for ph in range(2):
                        dst = bass.AP(
                            tensor=dst_dram.tensor,
                            offset=dst_dram.offset + (ct * 128 + ph * 64) * 256,
                            ap=[[256, 8], [8 * 256, 8], [1, 256]])
                        nc.scalar.dma_start(dst, xt[ph * 64:(ph + 1) * 64, :])
                else:
                    nc.scalar.dma_start(dst_dram[ct * 128:(ct + 1) * 128, :], xt[:])

            # ---- projections (six chains, PE-dense) ----

            # classifier weights + fp16 identity staged early so phase L
            # has no DMA dependency at the tail.
            wc_t = persist.tile([128, CT, NL], F16, name="wc_t")
            nc.sync.dma_start(wc_t[:], Wc_d.ap().rearrange("(c p) n -> p c n", p=128))
            ident16 = persist.tile([128, 128], F16, name="ident16")
            nc.scalar.copy(ident16[:], ident[:])

            def emit_tanh_ts(ct):
                # ts[pair=(il,j)] = tanh(At[j] + Bt[il] + Ct + bt)
                emit_tanh(ct, 2, 3, (IB, [[0, 8], [1, 32]]), (0, [[1, 8], [0, 32]]),
                          1, ts_dram, dup_order=False)

            def emit_tanh_hs(ct):
                # hs[pair=(il,j)] = tanh(Ah[il] + Bh[j] + Ch + bh)
                emit_tanh(ct, 0, 1, (0, [[1, 8], [0, 32]]), (IB, [[0, 8], [1, 32]]),
                          0, hs_dram, dup_order=True)

            # per-ct chain piece schedule: slot s of an even group emits one
            # piece for ct_next so PE/ACT/DVE work stays spread out.
            def emit_ct_piece(ct, s):
                if s == 0:
                    emit_ab_mini(2, ct)
                elif s == 1:
                    emit_ab_mini(3, ct)
                elif s == 2:
                    emit_tanh_ts(ct)
                elif s == 3:
                    emit_ab_mini(0, ct)
                elif s == 4:
                    emit_ab_mini(1, ct)
                elif s == 5:
                    emit_tanh_hs(ct)

            # head: ts-side C chain + ts pieces first, hs side after
            emit_c_chain(1, bt_t)
            for s in range(3):
                emit_ct_piece(0, s)
            emit_c_chain(0, bh_t)
            for s in range(3, 6):
                emit_ct_piece(0, s)

            # ---- phase M: main contraction over W_ext ----
            # k-tile (g, ib2, jb): partition p = di*16 + pj covers k-row
            # g*4096 + (ib2*8+di)*64 + jb*16 + pj.  Factor tiles per group:
            # hsdup[p, ib2, pair] = hs[g*64 + ib2*8 + p//16, pair]
            # tsdup[p, jb, pair]  = ts[g*64 + jb*16 + p%16, pair]
            def emit_hsdup_dma(hsdup, g, di):
                src = bass.AP(
                    tensor=hs_dram.tensor,
                    offset=hs_dram.offset + (g * 64 + di * 8) * 256,
                    ap=[[0, 16], [1, 8 * 256]])
                nc.scalar.dma_start(
                    hsdup[di * 16:(di + 1) * 16, :, :].rearrange(
                        "p l c -> p (l c)"), src)

            def emit_tsdup_load(tsdup, g):
                # load the 16 unique rows once, then log-double in SBUF
                src = bass.AP(
                    tensor=ts_dram.tensor,
                    offset=ts_dram.offset + g * 64 * 256,
                    ap=[[256, 16], [16 * 256, 4], [1, 256]])
                nc.sync.dma_start(tsdup[0:16, :, :], src)

            def emit_tsdup_double(tsdup, step):
                n = 16 << step
                nc.sync.dma_start(tsdup[n:2 * n, :, :], tsdup[0:n, :, :])

            def alloc_group():
                return (hsdupp.tile([128, 8, 256], F16, name="hsdup"),
                        tsdp.tile([128, 4, 256], F16, name="tsdup"))

            cur = alloc_group()
            for di in range(8):
                emit_hsdup_dma(cur[0], 0, di)
            emit_tsdup_load(cur[1], 0)
            for st in range(3):
                emit_tsdup_double(cur[1], st)

            wx_ch = None
            for g in range(G):
                nxt = alloc_group() if g + 1 < G else None
                ct_next = g // 2 + 1
                for ib2 in range(8):
                    # software-pipelined staging for group g+1
                    if nxt is not None:
                        if ib2 < 2:
                            for di in range(4 * ib2, 4 * ib2 + 4):
                                emit_hsdup_dma(nxt[0], g + 1, di)
                            if ib2 == 0:
                                emit_tsdup_load(nxt[1], g + 1)
                        elif ib2 < 5:
                            emit_tsdup_double(nxt[1], ib2 - 2)
                    # chain pieces for the next ct (even groups only)
                    if g % 2 == 0 and ct_next < CT and ib2 < 6:
                        emit_ct_piece(ct_next, ib2)

                    kt0 = g * 32 + ib2 * 4
                    bl4 = blp.tile([128, 4, 256], F16, name="bl4")
                    hs_b = bass.AP(tensor=cur[0].tensor,
                                   offset=cur[0].offset + ib2 * 256,
                                   ap=[cur[0].ap[0], [0, 4], [1, 256]])
                    nc.vector.tensor_tensor(bl4[:], hs_b, cur[1][:],
                                            op=mybir.AluOpType.mult)
                    for jb in range(4):
                        kt = kt0 + jb
                        if kt % WCH == 0:
                            wx_ch = wxp.tile([128, WCH * EMB], F16, name="wx_ch")
                            nc.sync.dma_start(
                                wx_ch[:], Wx_d.ap()[:, kt * EMB:(kt + WCH) * EMB])
                        kl = kt % WCH
                        for pt in range(PT):
                            lhsT = bl4[:, jb, pt * 128:(pt + 1) * 128]
                            nc.tensor.matmul(
                                ps_feat[pt][0][:], lhsT,
                                wx_ch[:, kl * EMB:kl * EMB + 512],
                                start=(kt == 0), stop=(kt == KT - 1))
                            nc.tensor.matmul(
                                ps_feat[pt][1][:], lhsT,
                                wx_ch[:, kl * EMB + 512:(kl + 1) * EMB],
                                start=(kt == 0), stop=(kt == KT - 1))
                cur = nxt

            # ---- phase L: bias, relu, layernorm, classifier ----
            for pt in range(PT):
                feat = persist.tile([128, EMB], F32, name=f"feat{pt}")
                nc.vector.tensor_tensor(feat[:, 0:512], ps_feat[pt][0][:],
                                        bx_b[:, 0:512], op=mybir.AluOpType.add)
                nc.vector.tensor_tensor(feat[:, 512:768], ps_feat[pt][1][:],
                                        bx_b[:, 512:768], op=mybir.AluOpType.add)
                nc.scalar.activation(feat[:], feat[:],
                                     mybir.ActivationFunctionType.Relu,
                                     bias=0.0, scale=1.0)

                stats = tmpp.tile([128, 3, 6], F32, name="stats")
                f_re = feat.rearrange("p (c f) -> p c f", c=3)
                for c in range(3):
                    nc.vector.bn_stats(stats[:, c, :], f_re[:, c, :])
                mv = tmpp.tile([128, 2], F32, name="mv")
                nc.vector.bn_aggr(mv[:], stats[:])
                sd = tmpp.tile([128, 1], F32, name="sd")
                nc.scalar.activation(sd[:], mv[:, 1:2],
                                     mybir.ActivationFunctionType.Sqrt,
                                     bias=eps_t[:], scale=1.0)
                rstd = tmpp.tile([128, 1], F32, name="rstd")
                nc.vector.reciprocal(rstd[:], sd[:])

                nc.vector.tensor_scalar(feat[:], feat[:], mv[:, 0:1], rstd[:],
                                        op0=mybir.AluOpType.subtract,
                                        op1=mybir.AluOpType.mult)
                nc.vector.tensor_tensor(feat[:], feat[:], lng_b[:],
                                        op=mybir.AluOpType.mult)
                ln = persist.tile([128, EMB], F16, name=f"ln{pt}")
                nc.vector.tensor_tensor(ln[:], feat[:], lnb_b[:],
                                        op=mybir.AluOpType.add)

                lnT = persist.tile([128, CT, 128], F16, name=f"lnT{pt}")
                for ct in range(CT):
                    ps_tr2 = psg.tile([128, 128], F16, name="gen")
                    nc.tensor.transpose(ps_tr2[:], ln[:, ct * 128:(ct + 1) * 128],
                                        ident16[:])
                    nc.scalar.copy(lnT[:, ct, :], ps_tr2[:])

                ps_lg = psg.tile([128, NL], F32, name="gen")
                for ct in range(CT):
                    nc.tensor.matmul(ps_lg[:], lnT[:, ct, :], wc_t[:, ct, :],
                                     start=(ct == 0), stop=(ct == CT - 1))
                out_sb = tmpp.tile([128, NL], F32, name="out_sb")
                nc.scalar.copy(out_sb[:], ps_lg[:])
                nc.scalar.dma_start(out_d.ap()[pt * 128:(pt + 1) * 128, :], out_sb[:])

    nc.compile()
    return nc


_NC_CACHE = []


def _get_module():
    if not _NC_CACHE:
        _NC_CACHE.append(_build_module())
    return _NC_CACHE[0]


def _build_inputs(seq, starts, ends, mention_mask, W_head, b_head, W_tail, b_tail,
                  W_ext, b_ext, ln_g, ln_b, W_cls):
    seq = np.asarray(seq, np.float32)
    starts = np.asarray(starts, np.int64)
    ends = np.asarray(ends, np.int64)
    mask = np.asarray(mention_mask, np.float32)

    # per-document entity selection matrix: ent = Sb^T @ seq[b]
    S_b = np.zeros((B, L, E), np.float32)
    denom = np.maximum(mask.sum(axis=2), 1.0)          # [B, E]
    w = mask * 0.5 / denom[:, :, None]                 # [B, E, M]
    for b in range(B):
        for e in range(E):
            np.add.at(S_b[b, :, e], starts[b, e] + 1, w[b, e])
            np.add.at(S_b[b, :, e], ends[b, e], w[b, e])

    cls_col = np.zeros((L, 1), np.float32)
    cls_col[0, 0] = 1.0

    shared = {
        "Wh": np.ascontiguousarray(np.asarray(W_head, np.float32).astype(np.float16)),
        "Wt": np.ascontiguousarray(np.asarray(W_tail, np.float32).astype(np.float16)),
        "bh": np.ascontiguousarray(np.asarray(b_head, np.float32).reshape(CT, 128).T),
        "bt": np.ascontiguousarray(np.asarray(b_tail, np.float32).reshape(CT, 128).T),
        # partition p = di*16+pj, kt = (g, ib2, jb); row k = g*4096 +
        # (ib2*8+di)*64 + jb*16 + pj
        "Wx": np.ascontiguousarray(
            np.asarray(W_ext).astype(np.float16)
            .reshape(G, 8, 8, 4, 16, EMB).transpose(2, 4, 0, 1, 3, 5)
            .reshape(128, KT * EMB)),
        "bx": np.ascontiguousarray(np.broadcast_to(np.asarray(b_ext, np.float32), (128, EMB))),
        "lng": np.ascontiguousarray(np.broadcast_to(np.asarray(ln_g, np.float32), (128, EMB))),
        "lnb": np.ascontiguousarray(np.broadcast_to(np.asarray(ln_b, np.float32), (128, EMB))),
        "Wc": np.ascontiguousarray(np.asarray(W_cls, np.float32).astype(np.float16)),
    }
    in_maps = []
    for core in range(N_CORES):
        b, ib = core // 4, core % 4
        S_core = np.concatenate(
            [S_b[b][:, ib * IB:(ib + 1) * IB], S_b[b], cls_col], axis=1)
        in_maps.append({
            "seq": np.ascontiguousarray(seq[b].astype(np.float16)),
            "S": np.ascontiguousarray(S_core.astype(np.float16)),
            **shared,
        })
    return in_maps


def kernel(**inputs) -> np.ndarray:
    nc = _get_module()
    in_maps = _build_inputs(**inputs)
    res = run_bass_kernel_spmd(nc, in_maps, core_ids=list(range(N_CORES)))
    outs = np.stack([res.results[c]["out"] for c in range(N_CORES)])  # [8,256,97]
    return outs.reshape(B, 4, IB, E, NL).reshape(B, E, E, NL)
